# revision 22
# baseline (speedup 1.0000x reference)
"""Trainium2 Bass kernel: SNN Leaky-Integrate-and-Fire layer.

For x [T=1024, N_IN=4096] f32 and W [N_OUT=4096, N_IN=4096] f32:
    cur = x @ W.T                                      # [T, N_OUT]
    mem_t = 0.9*mem_{t-1} + cur_t - (mem_{t-1} > 1)    # scan over T
    spk_t = (mem_t > 1)
returns (spk_rec, mem_rec), both [T, N_OUT] f32.

Sharding: N_OUT split across 8 NeuronCores (512 neurons each); x replicated.

Per-core plan (neurons on partitions, time along free dim):
  - matmul in fp32r (1 cycle/row at free size 512), single pass; k-major
    issue order so matmuls consume xT k-chunks in DMA arrival order; all
    8 PSUM banks hold the 4 o-tiles' [128, 1024] accumulators. Inputs use
    partition-major DRAM layouts (16KB contiguous per partition per chunk)
    for maximum DMA descriptor efficiency; the last k-tiles transfer in
    512KB pieces so the scan tail unblocks as early as possible.
  - scan decomposed as mem = A + B:
      A_t = 0.9*A_{t-1} + cur_t     one custom-DVE scan instr per o-tile
      B_t = 0.9*B_{t-1} - s_{t-1};  s_t = (B_t > 1 - A_t)
      mem_t = A_t + B_t             one custom-DVE LIF instr per o-tile
    (hand-written uop programs, 2 cycles/element, consume/bubble FSM with
    cross-element state in stage a-flops; model notes in _build_dve_uops)
  - only mem is DMA'd out; spk = (mem > 1) is host-side glue.
"""

import numpy as np

T = 1024
N_IN = 4096
N_OUT = 4096
N_CORES = 8
O_SHARD = N_OUT // N_CORES  # 512
KT = N_IN // 128  # 32 k-tiles
OT = O_SHARD // 128  # 4 o-tiles
BETA = 0.9
THRESHOLD = 1.0

_CACHE = {}

# ---------------------------------------------------------------------------
# Custom DVE uop programs (see dve_lif.py for the execution-model notes;
# inlined here so kernel.py is self-contained for the grading harness).
# ---------------------------------------------------------------------------


def _build_dve_uops():
    from concourse.dve_uop import (
        ENABLE,
        AluInp,
        AluOp,
        DelayInp,
        InpSel,
        OutPath,
        OutSel,
        Trigger,
        UopConfig,
    )

    def bubble(next_idx):
        u = UopConfig()
        u.trigger = (Trigger.COUNT, Trigger.NONE, Trigger.NONE)
        u.next_uop = (next_idx, 0, 0)
        u.repeat_count = 1
        return u

    def scan_lin():
        # out_t = A_t; A_t = c0*A_{t-1} + in0_t; A_{-1} = c1.
        seed = UopConfig()
        seed.enable_input(InpSel.CONST_1, 1)
        seed.datapath_config[0].pass_through_delay(0)
        seed.datapath_config[1].pass_through_delay(0)
        seed.datapath_config[2].enable_alu(AluOp.BYPASS, AluInp.PREV_DELAY_0)
        seed.datapath_config[2].alu_out_a_enable = ENABLE
        seed.trigger = (Trigger.COUNT, Trigger.NONE, Trigger.NONE)
        seed.next_uop = (1, 0, 0)
        seed.repeat_count = 1

        c = UopConfig()
        c.enable_input(InpSel.SRC_0, 1)  # lane 0: cur_t
        c.enable_input(InpSel.CONST_0, 2)  # lane 1: beta
        c.datapath_config[0].pass_through_delay(0, 1)
        c.datapath_config[1].enable_alu(
            AluOp.MULTIPLY, AluInp.NEXT_ALU_OUT_A, AluInp.PREV_DELAY_1
        ).pass_through_delay(0)
        c.datapath_config[2].enable_alu(
            AluOp.ADD, AluInp.PREV_ALU_OUT, AluInp.PREV_DELAY_0
        )
        c.datapath_config[2].alu_out_a_enable = ENABLE
        for st in range(3, 8):
            c.datapath_config[st].pass_through_alu()
        c.enable_output(OutSel.ALU_OUT, OutPath.WR0_LO)
        c.require_inp0 = ENABLE
        c.trigger = (Trigger.SRC_TENSOR_DONE, Trigger.COUNT, Trigger.NONE)
        c.next_uop = (0, 1, 0)
        c.repeat_count = 1
        return [seed, bubble(2), c]

    def lif_mem():
        # in0 = A_t, c0 = beta:
        #   th = 1 - A; u = beta*B; B' = u - s; s' = (B' > th); out = A + B'
        seed = UopConfig()
        seed.enable_input(InpSel.ZERO, 1)
        for st in range(3):
            seed.datapath_config[st].pass_through_delay(0)
        seed.datapath_config[3].enable_alu(AluOp.BYPASS, AluInp.PREV_DELAY_0)
        seed.datapath_config[3].alu_out_a_enable = ENABLE
        seed.datapath_config[3].pass_through_delay(0)
        seed.datapath_config[4].enable_alu(AluOp.BYPASS, AluInp.PREV_DELAY_0)
        seed.datapath_config[4].alu_out_a_enable = ENABLE
        seed.trigger = (Trigger.COUNT, Trigger.NONE, Trigger.NONE)
        seed.next_uop = (1, 0, 0)
        seed.repeat_count = 1

        c = UopConfig()
        c.enable_input(InpSel.SRC_0, 1)  # lane 0: A_t
        c.enable_input(InpSel.CONST_0, 2)  # lane 1: beta
        c.enable_input(InpSel.ONE_F32, 3)  # lane 2: 1.0
        c.datapath_config[0].pass_through_delay(0, 1, 2)
        c.datapath_config[1].enable_alu(
            AluOp.SUBTRACT, AluInp.PREV_DELAY_2, AluInp.PREV_DELAY_0
        ).pass_through_delay(0, 1)
        c.datapath_config[2].enable_alu(
            AluOp.MULTIPLY, AluInp.NEXT_ALU_OUT_A, AluInp.PREV_DELAY_1
        ).enable_delay_from_src(DelayInp.PREV_ALU_OUT, 2).pass_through_delay(0)
        c.datapath_config[3].enable_alu(
            AluOp.SUBTRACT, AluInp.PREV_ALU_OUT, AluInp.NEXT_ALU_OUT_A
        ).pass_through_delay(0, 2)
        c.datapath_config[3].alu_out_a_enable = ENABLE
        c.datapath_config[4].enable_alu(
            AluOp.IS_GT, AluInp.PREV_ALU_OUT, AluInp.PREV_DELAY_2
        ).enable_delay_from_src(DelayInp.PREV_ALU_OUT, 1).pass_through_delay(0)
        c.datapath_config[4].alu_out_a_enable = ENABLE
        c.datapath_config[5].enable_alu(
            AluOp.ADD, AluInp.PREV_DELAY_0, AluInp.PREV_DELAY_1
        )
        for st in range(6, 8):
            c.datapath_config[st].pass_through_alu()
        c.enable_output(OutSel.ALU_OUT, OutPath.WR0_LO)
        c.require_inp0 = ENABLE
        c.trigger = (Trigger.SRC_TENSOR_DONE, Trigger.COUNT, Trigger.NONE)
        c.next_uop = (0, 1, 0)
        c.repeat_count = 1
        return [seed, bubble(2), c]

    return scan_lin, lif_mem


def _ref_scan_lin(in0, in1, c0, c1, c2):
    out = np.empty_like(in0, dtype=np.float32)
    a = np.full((in0.shape[0],), c1, np.float32)
    for t in range(in0.shape[1]):
        a = np.float32(c0) * a + in0[:, t]
        out[:, t] = a
    return out


def _ref_lif_mem(in0, in1, c0, c1, c2):
    out = np.empty_like(in0, dtype=np.float32)
    b = np.zeros((in0.shape[0],), np.float32)
    s = np.zeros((in0.shape[0],), np.float32)
    for t in range(in0.shape[1]):
        th = np.float32(1.0) - in0[:, t]
        b = np.float32(c0) * b - s
        s = (b > th).astype(np.float32)
        out[:, t] = in0[:, t] + b
    return out


def _register_op(name, build_uops, ref):
    if name in _CACHE:
        return _CACHE[name]
    import concourse.dve_ops as dve_ops
    from concourse.dve_ops import DveOp
    from concourse.dve_spec import Spec, Src0
    from concourse.dve_uop import DveOpSpec

    class _RawDveOp(DveOp):
        def compile(self, ver):
            assert ver == "v3", f"{name} is v3/TRN2-only"
            key = (self.name, ver)
            if key not in _CACHE:
                _CACHE[key] = DveOpSpec(
                    name=self.name,
                    opcode=dve_ops.get_dve_sub_opcode(self.name),
                    uops=build_uops(),
                    rd1_en=False,
                )
            return _CACHE[key]

    op = _RawDveOp(
        name=name, spec=Spec(body=Src0, reference=ref), subdim=False, uops_sha={}
    )
    if name not in dve_ops._SUB_OPCODE_FOR_NAME:
        dve_ops.OPS.append(op)
        dve_ops._SUB_OPCODE_FOR_NAME[name] = (
            dve_ops._CUSTOM_DVE_ROW_BASE + len(dve_ops.OPS) - 1
        )
        dve_ops.CUSTOM_DVE_SPECS[name] = op.spec
    _CACHE[name] = op
    return op


def _get_ops():
    scan_lin, lif_mem = _build_dve_uops()
    return (
        _register_op("SCAN_LIN_ANT", scan_lin, _ref_scan_lin),
        _register_op("LIF_MEM_ANT", lif_mem, _ref_lif_mem),
    )


# ---------------------------------------------------------------------------
# Kernel build
# ---------------------------------------------------------------------------


def _build_nc():
    import concourse.bacc as bacc
    import concourse.mybir as mybir
    from concourse.tile import TileContext

    F32 = mybir.dt.float32
    F32R = mybir.dt.float32r
    scan_op, lif_op = _get_ops()

    nc = bacc.Bacc(target_bir_lowering=False)
    # partition-major layouts: row p holds all of partition p's data
    # contiguously, so each DMA chunk is a few 16KB-contiguous runs per
    # partition instead of many 4KB ones.
    xT_d = nc.dram_tensor("xT", [128, KT * T], F32R, kind="ExternalInput")
    WT_d = nc.dram_tensor("WT", [128, KT * OT * 128], F32R, kind="ExternalInput")
    mem_d = nc.dram_tensor("mem", [O_SHARD, T], F32, kind="ExternalOutput")

    with TileContext(nc) as tc:
        with (
            tc.tile_pool(name="sb", bufs=1) as sb,
            tc.tile_pool(name="ap", bufs=3) as ap,
            tc.tile_pool(name="psp", bufs=1, space="PSUM") as psp,
        ):
            xt = sb.tile([128, KT, T], F32R, name="xt")
            wt = sb.tile([128, KT, OT, 128], F32R, name="wt")
            xt_v = xT_d.rearrange("p (k t) -> p k t", t=T)
            wt_v = WT_d.rearrange("p (k o m) -> p k o m", o=OT, m=128)

            # DMA-in, k-ordered so matmuls stream behind the transfers;
            # first chunks extra-small so the first matmul starts early.
            nc.sync.dma_start(wt[:, 0:2], wt_v[:, 0:2])
            nc.sync.dma_start(xt[:, 0:1], xt_v[:, 0:1])
            nc.sync.dma_start(xt[:, 1:2], xt_v[:, 1:2])
            nc.sync.dma_start(wt[:, 2:4], wt_v[:, 2:4])
            nc.sync.dma_start(xt[:, 2:3], xt_v[:, 2:3])
            nc.sync.dma_start(xt[:, 3:4], xt_v[:, 3:4])
            for kc in range(4, KT - 4, 4):
                nc.sync.dma_start(wt[:, kc : kc + 4], wt_v[:, kc : kc + 4])
                nc.sync.dma_start(xt[:, kc : kc + 2], xt_v[:, kc : kc + 2])
                nc.sync.dma_start(
                    xt[:, kc + 2 : kc + 4], xt_v[:, kc + 2 : kc + 4]
                )
            # final k-tiles in 512KB pieces: the o-major tail (and with it
            # the whole scan chain) unblocks as early as possible
            nc.sync.dma_start(wt[:, KT - 4 : KT], wt_v[:, KT - 4 : KT])
            for k in range(KT - 4, KT):
                nc.sync.dma_start(xt[:, k : k + 1], xt_v[:, k : k + 1])

            ps = [
                psp.tile([128, T], F32, name=f"ps{o}", tag=f"ps{o}") for o in range(OT)
            ]

            def mm(k, o):
                for tl, tr in ((0, 512), (512, 1024)):
                    nc.tensor.matmul(
                        ps[o][:, tl:tr],
                        lhsT=wt[:, k, o, :],
                        rhs=xt[:, k, tl:tr],
                        start=(k == 0),
                        stop=(k == KT - 1),
                    )

            # k-major through K_TAIL, then o-major tail with scans emitted
            # right after each o's final matmul.
            K_TAIL = 28
            for k in range(K_TAIL):
                for o in range(OT):
                    mm(k, o)
            for o in range(OT):
                for k in range(K_TAIL, KT):
                    mm(k, o)
                A = ap.tile([128, T], F32, name="A")
                nc.vector._custom_dve(scan_op, out=A, in0=ps[o], s0=BETA, s1=0.0)
                nc.vector._custom_dve(lif_op, out=A, in0=A, s0=BETA)
                nc.sync.dma_start(mem_d[o * 128 : (o + 1) * 128, :], A)
    nc.finalize()
    return nc


def _get_nc():
    if "nc" not in _CACHE:
        _CACHE["nc"] = _build_nc()
    return _CACHE["nc"]


def run(x, W, mm_dtype_name=None, trace=False):
    from concourse.bass_utils import run_bass_kernel_spmd

    nc = _get_nc()
    x = np.asarray(x, dtype=np.float32)
    W = np.asarray(W, dtype=np.float32)
    xT = np.ascontiguousarray(
        x.T.reshape(KT, 128, T).transpose(1, 0, 2)
    ).reshape(128, KT * T)
    in_maps = []
    for c in range(N_CORES):
        WTc = np.ascontiguousarray(
            W[c * O_SHARD : (c + 1) * O_SHARD, :].T.reshape(KT, 128, O_SHARD)
            .transpose(1, 0, 2)
        ).reshape(128, KT * O_SHARD)
        in_maps.append({"xT": xT, "WT": WTc})
    res = run_bass_kernel_spmd(nc, in_maps, core_ids=list(range(N_CORES)), trace=trace)
    mem = np.concatenate([r["mem"] for r in res.results], axis=0).T
    mem = np.ascontiguousarray(mem)
    spk = (mem > THRESHOLD).astype(np.float32)
    return (spk, mem), res


def kernel(x, W):
    out, _ = run(x, W)
    return out


# revision 23
# speedup vs baseline: 1.0558x; 1.0558x over previous
"""Trainium2 Bass kernel: SNN Leaky-Integrate-and-Fire layer.

For x [T=1024, N_IN=4096] f32 and W [N_OUT=4096, N_IN=4096] f32:
    cur = x @ W.T                                      # [T, N_OUT]
    mem_t = 0.9*mem_{t-1} + cur_t - (mem_{t-1} > 1)    # scan over T
    spk_t = (mem_t > 1)
returns (spk_rec, mem_rec), both [T, N_OUT] f32.

Sharding: N_OUT split across 8 NeuronCores (512 neurons each); x replicated.

Per-core plan (neurons on partitions, time along free dim):
  - matmul in fp32r (1 cycle/row at free size 512), single pass; k-major
    issue order so matmuls consume xT k-chunks in DMA arrival order; all
    8 PSUM banks hold the 4 o-tiles' [128, 1024] accumulators. Inputs use
    partition-major DRAM layouts (16KB contiguous per partition per chunk)
    for maximum DMA descriptor efficiency; the last k-tiles transfer in
    512KB pieces so the scan tail unblocks as early as possible.
  - scan decomposed as mem = A + B:
      A_t = 0.9*A_{t-1} + cur_t     one custom-DVE scan instr per o-tile
      B_t = 0.9*B_{t-1} - s_{t-1};  s_t = (B_t > 1 - A_t)
      mem_t = A_t + B_t             one custom-DVE LIF instr per o-tile
    (hand-written uop programs, 2 cycles/element, consume/bubble FSM with
    cross-element state in stage a-flops; model notes in _build_dve_uops)
  - only mem is DMA'd out; spk = (mem > 1) is host-side glue.
"""

import numpy as np

T = 1024
N_IN = 4096
N_OUT = 4096
N_CORES = 8
O_SHARD = N_OUT // N_CORES  # 512
KT = N_IN // 128  # 32 k-tiles
OT = O_SHARD // 128  # 4 o-tiles
BETA = 0.9
THRESHOLD = 1.0

_CACHE = {}

# ---------------------------------------------------------------------------
# Custom DVE uop programs (see dve_lif.py for the execution-model notes;
# inlined here so kernel.py is self-contained for the grading harness).
# ---------------------------------------------------------------------------


def _build_dve_uops():
    from concourse.dve_uop import (
        ENABLE,
        AluInp,
        AluOp,
        DelayInp,
        InpSel,
        OutPath,
        OutSel,
        Trigger,
        UopConfig,
    )

    def bubble(next_idx):
        u = UopConfig()
        u.trigger = (Trigger.COUNT, Trigger.NONE, Trigger.NONE)
        u.next_uop = (next_idx, 0, 0)
        u.repeat_count = 1
        return u

    def scan_lin():
        # out_t = A_t; A_t = c0*A_{t-1} + in0_t; A_{-1} = c1.
        seed = UopConfig()
        seed.enable_input(InpSel.CONST_1, 1)
        seed.datapath_config[0].pass_through_delay(0)
        seed.datapath_config[1].pass_through_delay(0)
        seed.datapath_config[2].enable_alu(AluOp.BYPASS, AluInp.PREV_DELAY_0)
        seed.datapath_config[2].alu_out_a_enable = ENABLE
        seed.trigger = (Trigger.COUNT, Trigger.NONE, Trigger.NONE)
        seed.next_uop = (1, 0, 0)
        seed.repeat_count = 1

        c = UopConfig()
        c.enable_input(InpSel.SRC_0, 1)  # lane 0: cur_t
        c.enable_input(InpSel.CONST_0, 2)  # lane 1: beta
        c.datapath_config[0].pass_through_delay(0, 1)
        c.datapath_config[1].enable_alu(
            AluOp.MULTIPLY, AluInp.NEXT_ALU_OUT_A, AluInp.PREV_DELAY_1
        ).pass_through_delay(0)
        c.datapath_config[2].enable_alu(
            AluOp.ADD, AluInp.PREV_ALU_OUT, AluInp.PREV_DELAY_0
        )
        c.datapath_config[2].alu_out_a_enable = ENABLE
        for st in range(3, 8):
            c.datapath_config[st].pass_through_alu()
        c.enable_output(OutSel.ALU_OUT, OutPath.WR0_LO)
        c.require_inp0 = ENABLE
        c.trigger = (Trigger.SRC_TENSOR_DONE, Trigger.COUNT, Trigger.NONE)
        c.next_uop = (0, 1, 0)
        c.repeat_count = 1
        return [seed, bubble(2), c]

    def lif_mem():
        # in0 = A_t, c0 = beta:
        #   th = 1 - A; u = beta*B; B' = u - s; s' = (B' > th); out = A + B'
        seed = UopConfig()
        seed.enable_input(InpSel.ZERO, 1)
        for st in range(3):
            seed.datapath_config[st].pass_through_delay(0)
        seed.datapath_config[3].enable_alu(AluOp.BYPASS, AluInp.PREV_DELAY_0)
        seed.datapath_config[3].alu_out_a_enable = ENABLE
        seed.datapath_config[3].pass_through_delay(0)
        seed.datapath_config[4].enable_alu(AluOp.BYPASS, AluInp.PREV_DELAY_0)
        seed.datapath_config[4].alu_out_a_enable = ENABLE
        seed.trigger = (Trigger.COUNT, Trigger.NONE, Trigger.NONE)
        seed.next_uop = (1, 0, 0)
        seed.repeat_count = 1

        c = UopConfig()
        c.enable_input(InpSel.SRC_0, 1)  # lane 0: A_t
        c.enable_input(InpSel.CONST_0, 2)  # lane 1: beta
        c.enable_input(InpSel.ONE_F32, 3)  # lane 2: 1.0
        c.datapath_config[0].pass_through_delay(0, 1, 2)
        c.datapath_config[1].enable_alu(
            AluOp.SUBTRACT, AluInp.PREV_DELAY_2, AluInp.PREV_DELAY_0
        ).pass_through_delay(0, 1)
        c.datapath_config[2].enable_alu(
            AluOp.MULTIPLY, AluInp.NEXT_ALU_OUT_A, AluInp.PREV_DELAY_1
        ).enable_delay_from_src(DelayInp.PREV_ALU_OUT, 2).pass_through_delay(0)
        c.datapath_config[3].enable_alu(
            AluOp.SUBTRACT, AluInp.PREV_ALU_OUT, AluInp.NEXT_ALU_OUT_A
        ).pass_through_delay(0, 2)
        c.datapath_config[3].alu_out_a_enable = ENABLE
        c.datapath_config[4].enable_alu(
            AluOp.IS_GT, AluInp.PREV_ALU_OUT, AluInp.PREV_DELAY_2
        ).enable_delay_from_src(DelayInp.PREV_ALU_OUT, 1).pass_through_delay(0)
        c.datapath_config[4].alu_out_a_enable = ENABLE
        c.datapath_config[5].enable_alu(
            AluOp.ADD, AluInp.PREV_DELAY_0, AluInp.PREV_DELAY_1
        )
        for st in range(6, 8):
            c.datapath_config[st].pass_through_alu()
        c.enable_output(OutSel.ALU_OUT, OutPath.WR0_LO)
        c.require_inp0 = ENABLE
        c.trigger = (Trigger.SRC_TENSOR_DONE, Trigger.COUNT, Trigger.NONE)
        c.next_uop = (0, 1, 0)
        c.repeat_count = 1
        return [seed, bubble(2), c]

    return scan_lin, lif_mem


def _ref_scan_lin(in0, in1, c0, c1, c2):
    out = np.empty_like(in0, dtype=np.float32)
    a = np.full((in0.shape[0],), c1, np.float32)
    for t in range(in0.shape[1]):
        a = np.float32(c0) * a + in0[:, t]
        out[:, t] = a
    return out


def _ref_lif_mem(in0, in1, c0, c1, c2):
    out = np.empty_like(in0, dtype=np.float32)
    b = np.zeros((in0.shape[0],), np.float32)
    s = np.zeros((in0.shape[0],), np.float32)
    for t in range(in0.shape[1]):
        th = np.float32(1.0) - in0[:, t]
        b = np.float32(c0) * b - s
        s = (b > th).astype(np.float32)
        out[:, t] = in0[:, t] + b
    return out


def _register_op(name, build_uops, ref):
    if name in _CACHE:
        return _CACHE[name]
    import concourse.dve_ops as dve_ops
    from concourse.dve_ops import DveOp
    from concourse.dve_spec import Spec, Src0
    from concourse.dve_uop import DveOpSpec

    class _RawDveOp(DveOp):
        def compile(self, ver):
            assert ver == "v3", f"{name} is v3/TRN2-only"
            key = (self.name, ver)
            if key not in _CACHE:
                _CACHE[key] = DveOpSpec(
                    name=self.name,
                    opcode=dve_ops.get_dve_sub_opcode(self.name),
                    uops=build_uops(),
                    rd1_en=False,
                )
            return _CACHE[key]

    op = _RawDveOp(
        name=name, spec=Spec(body=Src0, reference=ref), subdim=False, uops_sha={}
    )
    if name not in dve_ops._SUB_OPCODE_FOR_NAME:
        dve_ops.OPS.append(op)
        dve_ops._SUB_OPCODE_FOR_NAME[name] = (
            dve_ops._CUSTOM_DVE_ROW_BASE + len(dve_ops.OPS) - 1
        )
        dve_ops.CUSTOM_DVE_SPECS[name] = op.spec
    _CACHE[name] = op
    return op


def _get_ops():
    scan_lin, lif_mem = _build_dve_uops()
    return (
        _register_op("SCAN_LIN_ANT", scan_lin, _ref_scan_lin),
        _register_op("LIF_MEM_ANT", lif_mem, _ref_lif_mem),
    )


# ---------------------------------------------------------------------------
# Kernel build
# ---------------------------------------------------------------------------


def _build_nc():
    import concourse.bacc as bacc
    import concourse.mybir as mybir
    from concourse.tile import TileContext

    F32 = mybir.dt.float32
    F32R = mybir.dt.float32r
    Op = mybir.AluOpType
    AF = mybir.ActivationFunctionType
    scan_op, lif_op = _get_ops()

    nc = bacc.Bacc(target_bir_lowering=False)
    # partition-major layouts: row p holds all of partition p's data
    # contiguously, so each DMA chunk is a few 16KB-contiguous runs per
    # partition instead of many 4KB ones.
    xT_d = nc.dram_tensor("xT", [128, 2 * KT * 512], F32R, kind="ExternalInput")
    WT_d = nc.dram_tensor("WT", [128, KT * OT * 128], F32R, kind="ExternalInput")
    mem_d = nc.dram_tensor("mem", [O_SHARD, T], F32, kind="ExternalOutput")

    with TileContext(nc) as tc:
        with (
            tc.tile_pool(name="sb", bufs=1) as sb,
            tc.tile_pool(name="ap", bufs=3) as ap,
            tc.tile_pool(name="psp", bufs=1, space="PSUM") as psp,
        ):
            xt = sb.tile([128, 2, KT, 512], F32R, name="xt")
            wt = sb.tile([128, KT, OT, 128], F32R, name="wt")
            beta_t = sb.tile([128, 512], F32, name="beta_t")
            nc.vector.memset(beta_t, BETA)
            xt_v = xT_d.rearrange("p (h k t) -> p h k t", h=2, t=512)
            wt_v = WT_d.rearrange("p (k o m) -> p k o m", o=OT, m=128)

            # Phase-1 stream: W + x-th0 (k-ordered, first chunks small);
            # phase-2 stream: x-th1, final k-tiles in small pieces so the
            # scan tail unblocks as early as possible.
            nc.sync.dma_start(wt[:, 0:2], wt_v[:, 0:2])
            nc.sync.dma_start(xt[:, 0, 0:2], xt_v[:, 0, 0:2])
            nc.sync.dma_start(wt[:, 2:4], wt_v[:, 2:4])
            nc.sync.dma_start(xt[:, 0, 2:4], xt_v[:, 0, 2:4])
            for kc in range(4, KT, 4):
                nc.sync.dma_start(wt[:, kc : kc + 4], wt_v[:, kc : kc + 4])
                nc.sync.dma_start(
                    xt[:, 0, kc : kc + 4], xt_v[:, 0, kc : kc + 4]
                )
            for kc in range(0, KT - 4, 4):
                nc.sync.dma_start(
                    xt[:, 1, kc : kc + 4], xt_v[:, 1, kc : kc + 4]
                )
            for k in range(KT - 4, KT):
                nc.sync.dma_start(xt[:, 1, k : k + 1], xt_v[:, 1, k : k + 1])

            ps = [
                psp.tile([128, T], F32, name=f"ps{o}", tag=f"ps{o}") for o in range(OT)
            ]

            def mm(k, o, th):
                nc.tensor.matmul(
                    ps[o][:, th * 512 : (th + 1) * 512],
                    lhsT=wt[:, k, o, :],
                    rhs=xt[:, th, k, :],
                    start=(k == 0),
                    stop=(k == KT - 1),
                )

            # phase 1: th0 matmuls stream behind the W + x-th0 DMA
            for k in range(KT):
                for o in range(OT):
                    mm(k, o, 0)
            # th0 A-scans in place in PSUM: hidden under the phase-2 stream
            for o in range(OT):
                nc.vector._custom_dve(
                    scan_op, out=ps[o][:, 0:512], in0=ps[o][:, 0:512],
                    s0=BETA, s1=0.0,
                )
            # phase 2: th1 matmuls; last k-tiles o-major so o0 unblocks first
            for k in range(KT - 4):
                for o in range(OT):
                    mm(k, o, 1)
            for o in range(OT):
                for k in range(KT - 4, KT):
                    mm(k, o, 1)
                A = ap.tile([128, T], F32, name="A")
                # th0 copy on Scalar, concurrent with the th1 scan on Vector
                nc.scalar.activation(A[:, 0:512], ps[o][:, 0:512], AF.Copy)
                nc.vector.tensor_tensor_scan(
                    out=A[:, 512:1024],
                    data0=beta_t,
                    data1=ps[o][:, 512:1024],
                    initial=ps[o][:, 511:512],
                    op0=Op.mult,
                    op1=Op.add,
                )
                nc.vector._custom_dve(lif_op, out=A, in0=A, s0=BETA)
                nc.sync.dma_start(mem_d[o * 128 : (o + 1) * 128, :], A)
    nc.finalize()
    return nc


def _get_nc():
    if "nc" not in _CACHE:
        _CACHE["nc"] = _build_nc()
    return _CACHE["nc"]


def run(x, W, mm_dtype_name=None, trace=False):
    from concourse.bass_utils import run_bass_kernel_spmd

    nc = _get_nc()
    x = np.asarray(x, dtype=np.float32)
    W = np.asarray(W, dtype=np.float32)
    xT = np.ascontiguousarray(
        x.T.reshape(KT, 128, 2, 512).transpose(1, 2, 0, 3)
    ).reshape(128, 2 * KT * 512)
    in_maps = []
    for c in range(N_CORES):
        WTc = np.ascontiguousarray(
            W[c * O_SHARD : (c + 1) * O_SHARD, :].T.reshape(KT, 128, O_SHARD)
            .transpose(1, 0, 2)
        ).reshape(128, KT * O_SHARD)
        in_maps.append({"xT": xT, "WT": WTc})
    res = run_bass_kernel_spmd(nc, in_maps, core_ids=list(range(N_CORES)), trace=trace)
    mem = np.concatenate([r["mem"] for r in res.results], axis=0).T
    mem = np.ascontiguousarray(mem)
    spk = (mem > THRESHOLD).astype(np.float32)
    return (spk, mem), res


def kernel(x, W):
    out, _ = run(x, W)
    return out
